# revision 6
# baseline (speedup 1.0000x reference)
"""Sliding-window (chunked) multi-head attention for Trainium2, 8-core SPMD.

Problem: B=1, S=8192, E=512, H=8 heads, Dh=64, window=1024 (half=512).
Reference math per window i (size 1024): keys span [i-512, i+1536).

Sharding: core c owns query window [1024c, 1024c+1024); it receives
x^T for the halo'd key range [1024c-512, 1024c+1536) (zero-padded at
the sequence edges) and computes q/k/v projections locally, windowed
softmax(q k^T / 8) v, and the output projection.  All compute layouts
are transposed ([E, seq]) so every matmul contracts over partitions;
the softmax denominator comes from a ones-augmented v (65th column).
bv is folded into an adjusted output-projection bias on the host
(attn rows sum to 1), so v needs no bias add on-chip.

Outputs are y^T shards [512, 1024] per core; the host transposes and
concatenates.
"""

import numpy as np
import ml_dtypes

import concourse.bass as bass
import concourse.tile as tile
from concourse import bacc, mybir
from concourse import bass_utils
from concourse.bass import ts

# ---- problem constants (hardcoded per contract) ----
S = 8192
E = 512
H = 8
DH = 64
NCORES = 8
SQ = 1024          # queries per core
SK = 2048          # halo'd keys per core
HALF = 512
SCALE = 0.125      # 1/sqrt(64)

F32 = mybir.dt.float32
F32R = mybir.dt.float32r
BF16 = mybir.dt.bfloat16

# ---- custom DVE op: exp(u/8) ~= (1 + c1 u + c2 u^2 + c3 u^3)^4 ----
# Fitted (Lawson minimax) on |u/8| <= 1.6; max rel err 7.2e-4.
_EC1 = 0.03126080224663743
_EC2 = 0.000493647595612354
_EC3 = 5.0261583805949835e-06


def _register_exp_op():
    from concourse import dve_ops as dops
    from concourse.dve_spec import Spec, Src0, One, C0, C1, C2, sq, lower
    from concourse.dve_uop import DveOpSpec

    name = "EXP4_ANT"
    for op in dops.OPS:
        if op.name == name:
            return op
    body = sq(sq(((C2 * Src0 + C1) * Src0 + C0) * Src0 + One))
    spec = Spec(body=body)
    shas = {}
    for ver in ("v3", "v4"):
        uops = lower(spec, ver=ver)
        shas[ver] = DveOpSpec(name=name, opcode=0, uops=uops, rd1_en=False).sha(ver)
    op = dops.DveOp(name, spec, subdim=False, uops_sha=shas)
    dops.OPS.append(op)
    dops.CUSTOM_DVE_SPECS[name] = spec
    dops._SUB_OPCODE_FOR_NAME[name] = dops._CUSTOM_DVE_ROW_BASE + len(dops.OPS) - 1
    assert max(dops._SUB_OPCODE_FOR_NAME.values()) < 0x20
    return op


def _build():
    """Build + compile the per-core Bass program (SPMD: same NEFF, 8 cores)."""
    exp_op = _register_exp_op()

    nc = bacc.Bacc("TRN2", target_bir_lowering=False, debug=False)

    xT_d = nc.dram_tensor("xT", [E, SK], BF16, kind="ExternalInput")
    W_d = {
        n: nc.dram_tensor(n, [E, E], BF16, kind="ExternalInput")
        for n in ("Wq", "Wk", "Wv", "Wo")
    }
    bq_d = nc.dram_tensor("bq", [E], F32, kind="ExternalInput")
    bk_d = nc.dram_tensor("bk", [E], F32, kind="ExternalInput")
    bo_d = nc.dram_tensor("bo_eff", [E], F32, kind="ExternalInput")
    mask_d = nc.dram_tensor("mask8", [128, SK // 128, H], BF16, kind="ExternalInput")
    yT_d = nc.dram_tensor("yT", [E, SQ], F32, kind="ExternalOutput")

    KT = 4   # E // 128 contraction tiles
    NKT = SK // 128  # 16 key tiles

    with tile.TileContext(nc) as tc:
        with (
            nc.allow_low_precision(reason="bf16/f32r attention kernel"),
            tc.tile_pool(name="singles", bufs=1) as singles,
            tc.tile_pool(name="exps", bufs=3) as exps,
            tc.tile_pool(name="recips", bufs=2) as recips,
            tc.tile_pool(name="bcs", bufs=2) as bcs,
            tc.tile_pool(name="ystage", bufs=3) as ystage,
        ):
            # ---- load everything ----
            xT_sb = singles.tile([128, KT, SK], BF16)
            for ke in range(KT):
                eng = nc.sync if ke % 2 == 0 else nc.gpsimd
                eng.dma_start(out=xT_sb[:, ke, :], in_=xT_d[ts(ke, 128), :])
            W_sb = {}
            for n, d in W_d.items():
                W_sb[n] = singles.tile([128, KT, E], BF16, tag=f"w_{n}", name=f"w_{n}")
                eng = nc.gpsimd if n in ("Wk", "Wo") else nc.sync
                eng.dma_start(out=W_sb[n], in_=d.ap().rearrange("(t p) j -> p t j", p=128))
            bq_sb = singles.tile([128, KT], F32, tag="bq")
            nc.sync.dma_start(out=bq_sb, in_=bq_d.ap().rearrange("(t p) -> p t", p=128))
            bk_sb = singles.tile([128, KT], F32, tag="bk")
            nc.sync.dma_start(out=bk_sb, in_=bk_d.ap().rearrange("(t p) -> p t", p=128))
            bo_sb = singles.tile([128, KT], F32, tag="bo")
            nc.sync.dma_start(out=bo_sb, in_=bo_d.ap().rearrange("(t p) -> p t", p=128))

            # v with ones column (from mask: 0 for padded keys)
            v_sb = singles.tile([128, NKT, H, DH + 1], BF16, tag="v")
            nc.sync.dma_start(out=v_sb[:, :, :, DH], in_=mask_d.ap())

            ones_f = singles.tile([1, DH], F32, tag="ones_f")
            nc.vector.memset(ones_f, 1.0)
            ones_r = singles.tile([1, DH], F32R, tag="ones_r")
            nc.vector.tensor_copy(out=ones_r, in_=ones_f)

            qT_sb = singles.tile([128, KT, SQ], BF16, tag="qT")
            kT_sb = singles.tile([128, KT, SK], BF16, tag="kT")
            outT_sb = singles.tile([128, KT, SQ], BF16, tag="outT")

            # ---- q/k/v projections ----
            with tc.tile_pool(name="pproj", bufs=4, space="PSUM") as pproj:
                for th in range(KT):
                    for qc in range(2):
                        ps = pproj.tile([128, 512], F32, tag="pp")
                        for ke in range(KT):
                            nc.tensor.matmul(
                                ps,
                                W_sb["Wq"][:, ke, ts(th, 128)],
                                xT_sb[:, ke, HALF + qc * 512:HALF + (qc + 1) * 512],
                                start=(ke == 0), stop=(ke == KT - 1),
                            )
                        nc.vector.tensor_scalar_add(
                            out=qT_sb[:, th, ts(qc, 512)], in0=ps, scalar1=bq_sb[:, th:th + 1]
                        )
                for th in range(KT):
                    for kc in range(4):
                        ps = pproj.tile([128, 512], F32, tag="pp")
                        for ke in range(KT):
                            nc.tensor.matmul(
                                ps,
                                W_sb["Wk"][:, ke, ts(th, 128)],
                                xT_sb[:, ke, ts(kc, 512)],
                                start=(ke == 0), stop=(ke == KT - 1),
                            )
                        nc.vector.tensor_scalar_add(
                            out=kT_sb[:, th, ts(kc, 512)], in0=ps, scalar1=bk_sb[:, th:th + 1]
                        )
                for st in range(NKT):
                    ps = pproj.tile([128, 512], F32, tag="pp")
                    for ke in range(KT):
                        nc.tensor.matmul(
                            ps,
                            xT_sb[:, ke, ts(st, 128)],
                            W_sb["Wv"][:, ke, :],
                            start=(ke == 0), stop=(ke == KT - 1),
                        )
                    nc.vector.tensor_copy(
                        out=v_sb[:, st, :, 0:DH],
                        in_=ps.rearrange("p (h d) -> p h d", h=H),
                    )

            # ---- windowed attention, one head at a time ----
            with (
                tc.tile_pool(name="pscore", bufs=2, space="PSUM") as pscore,
                tc.tile_pool(name="pav", bufs=2, space="PSUM") as pav,
            ):
                for h in range(H):
                    th, r0 = h // 2, 64 * (h % 2)
                    av_ps = pav.tile([DH + 1, SQ], F32, tag="av")
                    for kt in range(NKT):
                        s_ps = pscore.tile([128, SQ], F32, tag="s")
                        for qc in range(2):
                            nc.tensor.matmul(
                                s_ps[:, ts(qc, 512)],
                                kT_sb[r0:r0 + 64, th, ts(kt, 128)],
                                qT_sb[r0:r0 + 64, th, ts(qc, 512)],
                                start=True, stop=True,
                            )
                        e_sb = exps.tile([128, SQ], BF16, tag="e")
                        if kt % 3 == 2:
                            # custom DVE cubic^4 exp (coefficients fold in SCALE)
                            nc.vector._custom_dve(
                                exp_op, out=e_sb, in0=s_ps, s0=_EC1, s1=_EC2, imm2=_EC3
                            )
                        else:
                            nc.scalar.activation(
                                out=e_sb, in_=s_ps,
                                func=mybir.ActivationFunctionType.Exp, scale=SCALE,
                            )
                        for qc in range(2):
                            nc.tensor.matmul(
                                av_ps[:, ts(qc, 512)],
                                v_sb[:, kt, h, :],
                                e_sb[:, ts(qc, 512)],
                                start=(kt == 0), stop=(kt == NKT - 1),
                            )
                    # epilogue: normalize by the ones-column sums
                    # custom-DVE ops misread nonzero partition offsets: stage row 64
                    sums_st = recips.tile([1, SQ], F32, tag="st")
                    nc.vector.tensor_copy(out=sums_st, in_=av_ps[DH:DH + 1, :])
                    recip_f = recips.tile([1, SQ], F32, tag="rf")
                    nc.vector.reciprocal_approx_fast(out=recip_f, in_=sums_st)
                    recip = recips.tile([1, SQ], F32R, tag="r")
                    nc.vector.tensor_copy(out=recip, in_=recip_f)
                    bc_ps = pscore.tile([DH, SQ], F32, tag="s")
                    for qc in range(2):
                        nc.tensor.matmul(
                            bc_ps[:, ts(qc, 512)], ones_r, recip[0:1, ts(qc, 512)],
                            start=True, stop=True,
                        )
                    bc_sb = bcs.tile([DH, SQ], F32, tag="bc")
                    nc.vector.tensor_copy(out=bc_sb, in_=bc_ps)
                    for qc in range(2):
                        nc.vector.tensor_mul(
                            out=outT_sb[r0:r0 + 64, th, ts(qc, 512)],
                            in0=av_ps[0:DH, ts(qc, 512)],
                            in1=bc_sb[:, ts(qc, 512)],
                        )

            # ---- output projection ----
            with tc.tile_pool(name="py", bufs=4, space="PSUM") as py:
                for m in range(KT):
                    for qc in range(2):
                        ps = py.tile([128, 512], F32, tag="py")
                        for ke in range(KT):
                            nc.tensor.matmul(
                                ps,
                                W_sb["Wo"][:, ke, ts(m, 128)],
                                outT_sb[:, ke, ts(qc, 512)],
                                start=(ke == 0), stop=(ke == KT - 1),
                            )
                        yst = ystage.tile([128, 512], F32, tag="y")
                        nc.vector.tensor_scalar_add(out=yst, in0=ps, scalar1=bo_sb[:, m:m + 1])
                        nc.sync.dma_start(out=yT_d[ts(m, 128), ts(qc, 512)], in_=yst)

    nc.compile()
    return nc


_NC_CACHE = []


def _get_nc():
    if not _NC_CACHE:
        _NC_CACHE.append(_build())
    return _NC_CACHE[0]


def _prep_inputs(x, Wq, bq, Wk, bk, Wv, bv, Wo, bo):
    x = np.asarray(x, np.float32)
    xT_full = np.ascontiguousarray(x[0].T)  # [E, S]
    bo_eff = (np.asarray(bo, np.float64)
              + np.asarray(bv, np.float64) @ np.asarray(Wo, np.float64)).astype(np.float32)
    shared = {
        "Wq": np.ascontiguousarray(np.asarray(Wq, np.float32).astype(ml_dtypes.bfloat16)),
        "Wk": np.ascontiguousarray(np.asarray(Wk, np.float32).astype(ml_dtypes.bfloat16)),
        "Wv": np.ascontiguousarray(np.asarray(Wv, np.float32).astype(ml_dtypes.bfloat16)),
        "Wo": np.ascontiguousarray(np.asarray(Wo, np.float32).astype(ml_dtypes.bfloat16)),
        "bq": np.asarray(bq, np.float32),
        "bk": np.asarray(bk, np.float32),
        "bo_eff": bo_eff,
    }
    in_maps = []
    for c in range(NCORES):
        g0 = 1024 * c - HALF
        xT_halo = np.zeros((E, SK), np.float32)
        lo, hi = max(0, g0), min(S, g0 + SK)
        xT_halo[:, lo - g0:hi - g0] = xT_full[:, lo:hi]
        mask = np.zeros((SK, H), np.float32)
        mask[lo - g0:hi - g0, :] = 1.0
        mask = np.ascontiguousarray(mask.reshape(SK // 128, 128, H).transpose(1, 0, 2))
        m = dict(shared)
        m["xT"] = xT_halo.astype(ml_dtypes.bfloat16)
        m["mask8"] = mask.astype(ml_dtypes.bfloat16)
        in_maps.append(m)
    return in_maps


def run(inputs: dict, trace: bool = False):
    nc = _get_nc()
    in_maps = _prep_inputs(**inputs)
    res = bass_utils.run_bass_kernel_spmd(
        nc, in_maps, core_ids=list(range(NCORES)), trace=trace
    )
    y = np.concatenate([r["yT"].T for r in res.results], axis=0)[None]
    return np.ascontiguousarray(y.astype(np.float32)), res


def kernel(**inputs) -> np.ndarray:
    y, _ = run(inputs, trace=False)
    return y


# revision 7
# speedup vs baseline: 1.1662x; 1.1662x over previous
"""Sliding-window (chunked) multi-head attention for Trainium2, 8-core SPMD.

Problem: B=1, S=8192, E=512, H=8 heads, Dh=64, window=1024 (half=512).
Reference math per window i (size 1024): keys span [i-512, i+1536).

Sharding: core c owns query window [1024c, 1024c+1024); it receives
x^T for the halo'd key range [1024c-512, 1024c+1536) (zero-padded at
the sequence edges) and computes q/k/v projections locally, windowed
softmax(q k^T / 8) v, and the output projection.  All compute layouts
are transposed ([E, seq]) so every matmul contracts over partitions;
the softmax denominator comes from a ones-augmented v (65th column).
bv is folded into an adjusted output-projection bias on the host
(attn rows sum to 1), so v needs no bias add on-chip.

Outputs are y^T shards [512, 1024] per core; the host transposes and
concatenates.
"""

import numpy as np
import ml_dtypes

import concourse.bass as bass
import concourse.tile as tile
from concourse import bacc, mybir
from concourse import bass_utils
from concourse.bass import ts

# ---- problem constants (hardcoded per contract) ----
S = 8192
E = 512
H = 8
DH = 64
NCORES = 8
SQ = 1024          # queries per core
SK = 2048          # halo'd keys per core
HALF = 512
SCALE = 0.125      # 1/sqrt(64)

F32 = mybir.dt.float32
F32R = mybir.dt.float32r
BF16 = mybir.dt.bfloat16

# ---- custom DVE op: exp(u/8) ~= (1 + c1 u + c2 u^2 + c3 u^3)^4 ----
# Fitted (Lawson minimax) on |u/8| <= 1.6; max rel err 7.2e-4.
_EC1 = 0.03126080224663743
_EC2 = 0.000493647595612354
_EC3 = 5.0261583805949835e-06


def _register_exp_op():
    from concourse import dve_ops as dops
    from concourse.dve_spec import Spec, Src0, One, C0, C1, C2, sq, lower
    from concourse.dve_uop import DveOpSpec

    name = "EXP4_ANT"
    for op in dops.OPS:
        if op.name == name:
            return op
    body = sq(sq(((C2 * Src0 + C1) * Src0 + C0) * Src0 + One))
    spec = Spec(body=body)
    shas = {}
    for ver in ("v3", "v4"):
        uops = lower(spec, ver=ver)
        shas[ver] = DveOpSpec(name=name, opcode=0, uops=uops, rd1_en=False).sha(ver)
    op = dops.DveOp(name, spec, subdim=False, uops_sha=shas)
    dops.OPS.append(op)
    dops.CUSTOM_DVE_SPECS[name] = spec
    dops._SUB_OPCODE_FOR_NAME[name] = dops._CUSTOM_DVE_ROW_BASE + len(dops.OPS) - 1
    assert max(dops._SUB_OPCODE_FOR_NAME.values()) < 0x20
    return op


def _build():
    """Build + compile the per-core Bass program (SPMD: same NEFF, 8 cores)."""
    exp_op = _register_exp_op()

    nc = bacc.Bacc("TRN2", target_bir_lowering=False, debug=False)

    xT_d = nc.dram_tensor("xT", [E, SK], BF16, kind="ExternalInput")
    W_d = {
        n: nc.dram_tensor(n, [E, E], BF16, kind="ExternalInput")
        for n in ("Wq", "Wk", "Wv", "Wo")
    }
    bq_d = nc.dram_tensor("bq", [E], F32, kind="ExternalInput")
    bk_d = nc.dram_tensor("bk", [E], F32, kind="ExternalInput")
    bo_d = nc.dram_tensor("bo_eff", [E], F32, kind="ExternalInput")
    mask_d = nc.dram_tensor("mask8", [128, SK // 128, H], BF16, kind="ExternalInput")
    yT_d = nc.dram_tensor("yT", [E, SQ], F32, kind="ExternalOutput")

    KT = 4   # E // 128 contraction tiles
    NKT = SK // 128  # 16 key tiles

    with tile.TileContext(nc) as tc:
        with (
            nc.allow_low_precision(reason="bf16/f32r attention kernel"),
            tc.tile_pool(name="singles", bufs=1) as singles,
            tc.tile_pool(name="exps", bufs=3) as exps,
            tc.tile_pool(name="recips", bufs=2) as recips,
            tc.tile_pool(name="avus", bufs=2) as avus,
            tc.tile_pool(name="bcs", bufs=2) as bcs,
            tc.tile_pool(name="ystage", bufs=3) as ystage,
        ):
            # ---- load everything ----
            xT_sb = singles.tile([128, KT, SK], BF16)
            for ke in range(KT):
                eng = nc.sync if ke % 2 == 0 else nc.gpsimd
                eng.dma_start(out=xT_sb[:, ke, :], in_=xT_d[ts(ke, 128), :])
            W_sb = {}
            for n, d in W_d.items():
                W_sb[n] = singles.tile([128, KT, E], BF16, tag=f"w_{n}", name=f"w_{n}")
                eng = nc.gpsimd if n in ("Wk", "Wo") else nc.sync
                eng.dma_start(out=W_sb[n], in_=d.ap().rearrange("(t p) j -> p t j", p=128))
            bq_sb = singles.tile([128, KT], F32, tag="bq")
            nc.sync.dma_start(out=bq_sb, in_=bq_d.ap().rearrange("(t p) -> p t", p=128))
            bk_sb = singles.tile([128, KT], F32, tag="bk")
            nc.sync.dma_start(out=bk_sb, in_=bk_d.ap().rearrange("(t p) -> p t", p=128))
            bo_sb = singles.tile([128, KT], F32, tag="bo")
            nc.sync.dma_start(out=bo_sb, in_=bo_d.ap().rearrange("(t p) -> p t", p=128))

            # v with ones column (from mask: 0 for padded keys)
            v_sb = singles.tile([128, NKT, H, DH + 1], BF16, tag="v")
            nc.sync.dma_start(out=v_sb[:, :, :, DH], in_=mask_d.ap())

            ones_f = singles.tile([1, DH], F32, tag="ones_f")
            nc.vector.memset(ones_f, 1.0)
            ones_r = singles.tile([1, DH], F32R, tag="ones_r")
            nc.vector.tensor_copy(out=ones_r, in_=ones_f)

            qT_sb = singles.tile([128, KT, SQ], BF16, tag="qT")
            kT_sb = singles.tile([128, KT, SK], BF16, tag="kT")
            outT_sb = singles.tile([128, KT, SQ], BF16, tag="outT")

            # ---- q/k/v projections ----
            with tc.tile_pool(name="pproj", bufs=4, space="PSUM") as pproj:
                for th in range(KT):
                    for qc in range(2):
                        ps = pproj.tile([128, 512], F32, tag="pp")
                        for ke in range(KT):
                            nc.tensor.matmul(
                                ps,
                                W_sb["Wq"][:, ke, ts(th, 128)],
                                xT_sb[:, ke, HALF + qc * 512:HALF + (qc + 1) * 512],
                                start=(ke == 0), stop=(ke == KT - 1),
                            )
                        nc.vector.tensor_scalar_add(
                            out=qT_sb[:, th, ts(qc, 512)], in0=ps, scalar1=bq_sb[:, th:th + 1]
                        )
                for th in range(KT):
                    for kc in range(4):
                        ps = pproj.tile([128, 512], F32, tag="pp")
                        for ke in range(KT):
                            nc.tensor.matmul(
                                ps,
                                W_sb["Wk"][:, ke, ts(th, 128)],
                                xT_sb[:, ke, ts(kc, 512)],
                                start=(ke == 0), stop=(ke == KT - 1),
                            )
                        nc.vector.tensor_scalar_add(
                            out=kT_sb[:, th, ts(kc, 512)], in0=ps, scalar1=bk_sb[:, th:th + 1]
                        )
                for st in range(NKT):
                    ps = pproj.tile([128, 512], F32, tag="pp")
                    for ke in range(KT):
                        nc.tensor.matmul(
                            ps,
                            xT_sb[:, ke, ts(st, 128)],
                            W_sb["Wv"][:, ke, :],
                            start=(ke == 0), stop=(ke == KT - 1),
                        )
                    nc.vector.tensor_copy(
                        out=v_sb[:, st, :, 0:DH],
                        in_=ps.rearrange("p (h d) -> p h d", h=H),
                    )

            # ---- windowed attention, one head at a time ----
            with (
                tc.tile_pool(name="pscore", bufs=2, space="PSUM") as pscore,
                tc.tile_pool(name="pav", bufs=2, space="PSUM") as pav,
            ):
                for h in range(H):
                    th, r0 = h // 2, 64 * (h % 2)
                    av_ps = pav.tile([DH + 1, SQ], F32, tag="av")
                    for kt in range(NKT):
                        s_ps = pscore.tile([128, SQ], F32, tag="s")
                        for qc in range(2):
                            nc.tensor.matmul(
                                s_ps[:, ts(qc, 512)],
                                kT_sb[r0:r0 + 64, th, ts(kt, 128)],
                                qT_sb[r0:r0 + 64, th, ts(qc, 512)],
                                start=True, stop=True,
                            )
                        e_sb = exps.tile([128, SQ], BF16, tag="e")
                        if kt % 3 == 2:
                            # custom DVE cubic^4 exp (coefficients fold in SCALE)
                            nc.vector._custom_dve(
                                exp_op, out=e_sb, in0=s_ps, s0=_EC1, s1=_EC2, imm2=_EC3
                            )
                        else:
                            nc.scalar.activation(
                                out=e_sb, in_=s_ps,
                                func=mybir.ActivationFunctionType.Exp, scale=SCALE,
                            )
                        for qc in range(2):
                            nc.tensor.matmul(
                                av_ps[:, ts(qc, 512)],
                                v_sb[:, kt, h, :],
                                e_sb[:, ts(qc, 512)],
                                start=(kt == 0), stop=(kt == NKT - 1),
                            )
                    # epilogue: normalize by the ones-column sums
                    # evacuate the accumulator in one op to free its PSUM slot,
                    # then normalize entirely off SBUF (custom-DVE ops misread
                    # nonzero partition offsets, hence the row-64 staging copy)
                    avu = avus.tile([DH + 1, SQ], F32, tag="avu")
                    nc.vector.tensor_copy(out=avu, in_=av_ps)
                    sums_st = recips.tile([1, SQ], F32, tag="st")
                    nc.vector.tensor_copy(out=sums_st, in_=avu[DH:DH + 1, :])
                    recip_f = recips.tile([1, SQ], F32, tag="rf")
                    nc.vector.reciprocal_approx_fast(out=recip_f, in_=sums_st)
                    recip = recips.tile([1, SQ], F32R, tag="r")
                    nc.vector.tensor_copy(out=recip, in_=recip_f)
                    bc_ps = pscore.tile([DH, SQ], F32, tag="s")
                    for qc in range(2):
                        nc.tensor.matmul(
                            bc_ps[:, ts(qc, 512)], ones_r, recip[0:1, ts(qc, 512)],
                            start=True, stop=True,
                        )
                    for qc in range(2):
                        nc.vector.tensor_mul(
                            out=outT_sb[r0:r0 + 64, th, ts(qc, 512)],
                            in0=avu[0:DH, ts(qc, 512)],
                            in1=bc_ps[:, ts(qc, 512)],
                        )

            # ---- output projection ----
            with tc.tile_pool(name="py", bufs=4, space="PSUM") as py:
                for m in range(KT):
                    for qc in range(2):
                        ps = py.tile([128, 512], F32, tag="py")
                        for ke in range(KT):
                            nc.tensor.matmul(
                                ps,
                                W_sb["Wo"][:, ke, ts(m, 128)],
                                outT_sb[:, ke, ts(qc, 512)],
                                start=(ke == 0), stop=(ke == KT - 1),
                            )
                        yst = ystage.tile([128, 512], F32, tag="y")
                        nc.vector.tensor_scalar_add(out=yst, in0=ps, scalar1=bo_sb[:, m:m + 1])
                        nc.sync.dma_start(out=yT_d[ts(m, 128), ts(qc, 512)], in_=yst)

    nc.compile()
    return nc


_NC_CACHE = []


def _get_nc():
    if not _NC_CACHE:
        _NC_CACHE.append(_build())
    return _NC_CACHE[0]


def _prep_inputs(x, Wq, bq, Wk, bk, Wv, bv, Wo, bo):
    x = np.asarray(x, np.float32)
    xT_full = np.ascontiguousarray(x[0].T)  # [E, S]
    bo_eff = (np.asarray(bo, np.float64)
              + np.asarray(bv, np.float64) @ np.asarray(Wo, np.float64)).astype(np.float32)
    shared = {
        "Wq": np.ascontiguousarray(np.asarray(Wq, np.float32).astype(ml_dtypes.bfloat16)),
        "Wk": np.ascontiguousarray(np.asarray(Wk, np.float32).astype(ml_dtypes.bfloat16)),
        "Wv": np.ascontiguousarray(np.asarray(Wv, np.float32).astype(ml_dtypes.bfloat16)),
        "Wo": np.ascontiguousarray(np.asarray(Wo, np.float32).astype(ml_dtypes.bfloat16)),
        "bq": np.asarray(bq, np.float32),
        "bk": np.asarray(bk, np.float32),
        "bo_eff": bo_eff,
    }
    in_maps = []
    for c in range(NCORES):
        g0 = 1024 * c - HALF
        xT_halo = np.zeros((E, SK), np.float32)
        lo, hi = max(0, g0), min(S, g0 + SK)
        xT_halo[:, lo - g0:hi - g0] = xT_full[:, lo:hi]
        mask = np.zeros((SK, H), np.float32)
        mask[lo - g0:hi - g0, :] = 1.0
        mask = np.ascontiguousarray(mask.reshape(SK // 128, 128, H).transpose(1, 0, 2))
        m = dict(shared)
        m["xT"] = xT_halo.astype(ml_dtypes.bfloat16)
        m["mask8"] = mask.astype(ml_dtypes.bfloat16)
        in_maps.append(m)
    return in_maps


def run(inputs: dict, trace: bool = False):
    nc = _get_nc()
    in_maps = _prep_inputs(**inputs)
    res = bass_utils.run_bass_kernel_spmd(
        nc, in_maps, core_ids=list(range(NCORES)), trace=trace
    )
    y = np.concatenate([r["yT"].T for r in res.results], axis=0)[None]
    return np.ascontiguousarray(y.astype(np.float32)), res


def kernel(**inputs) -> np.ndarray:
    y, _ = run(inputs, trace=False)
    return y
